# revision 1
# baseline (speedup 1.0000x reference)
"""Trainium2 Bass kernel for the LGP-instruction module (read -> op bank -> write).

Data-parallel over batch: core b computes x[b] (2048, 4096).
Device pipeline per core:
  phase 1: valuesT[C, T] = sum_vt rw_tile[vt].T @ xT_tile[vt]   (PSUM, 4 banks)
  phase 2: per T-chunk of 512:
     h_i = W_i.T @ valuesT  (PSUM) -> ACT f_i(h + b_i) -> DVE weighted-accumulate
     out[Tsub, V] = accT.T @ wwT  -> DVE copy -> DMA store
Host prep: read_w softmax, write_w*out_scale transpose, x[b].T layout.
Matmuls run as float32r (fp32 bits, full-rate PE streaming).
"""
import sys
import numpy as np

if '/opt/trn_rl_repo' not in sys.path:
    sys.path.insert(0, '/opt/trn_rl_repo')

B, T, V, C, NOPS = 8, 2048, 4096, 128, 8
NCORES = 8
NV = V // 128     # 32 v-tiles
NTC = T // 512    # 4 T-chunks

_CACHE = {}
LAST_RESULT = None


def _build(pre, post):
    from concourse import bass, bacc, tile, mybir
    f32, f32r = mybir.dt.float32, mybir.dt.float32r
    AF = mybir.ActivationFunctionType
    ts = bass.ts
    FUNCS = [AF.Identity, AF.Relu, AF.Gelu, AF.Square,
             AF.Identity, AF.Abs, AF.Tanh, AF.Sigmoid]

    nc = bacc.Bacc("TRN2", target_bir_lowering=False, debug=False,
                   num_devices=NCORES)
    xT = nc.dram_tensor("xT", [V, T], f32r, kind="ExternalInput")
    rw = nc.dram_tensor("rw", [V, C], f32r, kind="ExternalInput")
    wwT = nc.dram_tensor("wwT", [C, V], f32r, kind="ExternalInput")
    opw = nc.dram_tensor("opw", [NOPS, C, C], f32r, kind="ExternalInput")
    opb = nc.dram_tensor("opb", [C, NOPS], f32, kind="ExternalInput")
    out = nc.dram_tensor("out", [T, V], f32, kind="ExternalOutput")

    NBLK = 4          # xT load blocks per T-chunk
    VB = NV // NBLK   # 8 v-tiles per block

    # xT viewed as [p, vtile, t]
    xTr = xT.ap().rearrange("(vb p) t -> p vb t", p=128)

    with tile.TileContext(nc) as tc:
        with tc.tile_pool(name="const", bufs=1) as constp, \
             tc.tile_pool(name="xt", bufs=6) as xtp, \
             tc.tile_pool(name="vals_ps", bufs=2, space="PSUM") as vpsp, \
             tc.tile_pool(name="vals_sb", bufs=2) as vsbp, \
             tc.tile_pool(name="h_ps", bufs=3, space="PSUM") as hpsp, \
             tc.tile_pool(name="t_sb", bufs=3) as tp, \
             tc.tile_pool(name="acc", bufs=2) as accp, \
             tc.tile_pool(name="out_ps", bufs=3, space="PSUM") as opsp, \
             tc.tile_pool(name="out_sb", bufs=2) as osbp:

            rw_t = constp.tile([128, NV, C], f32r)
            nc.sync.dma_start(rw_t[:], rw.ap().rearrange("(vt p) c -> p vt c", p=128))
            wwT_t = constp.tile([C, V], f32r)
            nc.sync.dma_start(wwT_t[:], wwT.ap())
            opw_t = constp.tile([C, NOPS, C], f32r)
            nc.sync.dma_start(opw_t[:], opw.ap().rearrange("i p c -> p i c"))
            opb_t = constp.tile([C, NOPS], f32)
            nc.sync.dma_start(opb_t[:], opb.ap())

            for tcn in range(NTC):
                # read: accumulate over all V into one psum bank
                values = vpsp.tile([128, 512], f32)
                for blk in range(NBLK):
                    xt = xtp.tile([128, VB, 512], f32r)
                    nc.sync.dma_start(
                        xt[:], xTr[:, ts(blk, VB), ts(tcn, 512)])
                    for j in range(VB):
                        vt = blk * VB + j
                        nc.tensor.matmul(values[:], rw_t[:, vt, :], xt[:, j, :],
                                         start=(vt == 0), stop=(vt == NV - 1))
                vals = vsbp.tile([128, 512], f32r)
                nc.vector.tensor_copy(vals[:], values[:])

                # op bank
                acc = accp.tile([128, 512], f32r)
                for i in range(NOPS):
                    h = hpsp.tile([128, 512], f32)
                    nc.tensor.matmul(h[:], opw_t[:, i, :], vals[:],
                                     start=True, stop=True)
                    if i == 0:
                        nc.scalar.activation(acc[:], h[:], FUNCS[0],
                                             bias=opb_t[:, 0:1], scale=pre[0])
                    else:
                        t = tp.tile([128, 512], f32r)
                        nc.scalar.activation(t[:], h[:], FUNCS[i],
                                             bias=opb_t[:, i:i + 1], scale=pre[i])
                        nc.vector.scalar_tensor_tensor(
                            acc[:], t[:], post[i], acc[:],
                            op0=mybir.AluOpType.mult, op1=mybir.AluOpType.add)

                # write: out rows, stores on SWDGE so loads never queue behind them
                for sub in range(4):
                    osb = osbp.tile([128, V], f32)
                    for nn in range(8):
                        ops_ = opsp.tile([128, 512], f32)
                        nc.tensor.matmul(ops_[:], acc[:, ts(sub, 128)],
                                         wwT_t[:, ts(nn, 512)],
                                         start=True, stop=True)
                        idx = (tcn * 4 + sub) * 8 + nn
                        if idx % 9 < 2:   # ~2/9 of psum-drain copies go to ACT
                            nc.scalar.copy(osb[:, ts(nn, 512)], ops_[:])
                        else:
                            nc.vector.tensor_copy(osb[:, ts(nn, 512)], ops_[:])
                    nc.gpsimd.dma_start(out.ap()[ts(tcn * 4 + sub, 128), :], osb[:])
    nc.compile()
    return nc


def _softmax(x, axis):
    x = np.asarray(x, np.float32)
    m = x.max(axis=axis, keepdims=True)
    e = np.exp(x - m)
    return e / e.sum(axis=axis, keepdims=True)


def kernel(x, basis, read_coeffs, write_coeffs, op_logits, op_weights,
           op_biases, out_scale):
    global LAST_RESULT
    from concourse.bass_utils import run_bass_kernel_spmd

    x = np.asarray(x, np.float32)
    basis = np.asarray(basis, np.float32)
    read_coeffs = np.asarray(read_coeffs, np.float32)
    write_coeffs = np.asarray(write_coeffs, np.float32)
    op_logits = np.asarray(op_logits, np.float32)
    op_weights = np.asarray(op_weights, np.float32)
    op_biases = np.asarray(op_biases, np.float32)
    out_scale = np.float32(out_scale)

    read_w = _softmax(basis @ read_coeffs.T, axis=0)               # (V, C)
    wwT = np.ascontiguousarray((basis @ write_coeffs.T).T) * out_scale  # (C, V)
    w = _softmax(op_logits, axis=0).astype(np.float64)

    # fold the mixture weight into ACT scale/bias where the nonlinearity allows
    #   i: 0 ident, 1 relu, 2 gelu, 3 square, 4 neg, 5 abs, 6 tanh, 7 sigmoid
    pre = [w[0], w[1], 1.0, np.sqrt(w[3]), -w[4], w[5], 1.0, 1.0]
    post = [1.0, 1.0, w[2], 1.0, 1.0, 1.0, w[6], w[7]]
    pre = [float(v) for v in pre]
    post = [float(v) for v in post]

    key = tuple(pre) + tuple(post)
    if key not in _CACHE:
        _CACHE[key] = _build(pre, post)
    nc = _CACHE[key]

    opb = (op_biases.T * np.array(pre, np.float64)[None, :]).astype(np.float32)
    # gelu/tanh/sigmoid biases enter before the nonlinearity unscaled
    for i in (2, 6, 7):
        opb[:, i] = op_biases[i]

    shared = {
        "rw": read_w,
        "wwT": wwT.astype(np.float32),
        "opw": op_weights,
        "opb": np.ascontiguousarray(opb),
    }
    in_maps = []
    for b in range(B):
        m = dict(shared)
        m["xT"] = np.ascontiguousarray(x[b].T)
        in_maps.append(m)

    res = run_bass_kernel_spmd(nc, in_maps, core_ids=list(range(NCORES)))
    LAST_RESULT = res
    out = np.empty((B, T, V), np.float32)
    for b in range(B):
        out[b] = res.results[b]["out"]
    return out



# revision 2
# speedup vs baseline: 1.6743x; 1.6743x over previous
"""Trainium2 Bass kernel for the LGP-instruction module (read -> op bank -> write).

Data-parallel over batch: core b computes x[b] (2048, 4096).
All HBM traffic in bf16 (x, params, out) -- the problem is memory-bound, so
halving bytes halves the roofline; rel-err budget (2e-2) easily covers bf16.

Device pipeline per core, per T-chunk of 512:
  phase 1: values[C, 512] = sum_vt rw[vt].T @ x_tile[vt]    (PSUM f32, bf16 MMs)
  phase 2: h_i = W_i.T @ vals (PSUM) -> ACT f_i(pre*h + b_i) -> DVE accumulate (f32)
  phase 3: out[128T, V] = acc_bf.T @ wwT  (PSUM) -> drain bf16 -> SWDGE store

Host prep: read_w softmax, write_w*out_scale transpose, x pre-tiled to the
exact SBUF layout ([128p, chunk, vtile, t]) in bf16 so every load is one
contiguous 2MB descriptor set. Output returned bf16, upcast on host.
"""
import sys
import numpy as np

if '/opt/trn_rl_repo' not in sys.path:
    sys.path.insert(0, '/opt/trn_rl_repo')

B, T, V, C, NOPS = 8, 2048, 4096, 128, 8
NCORES = 8
NV = V // 128     # 32 v-tiles
NTC = T // 512    # 4 T-chunks
NBLK = 2          # x load blocks per T-chunk (2MB each)
VB = NV // NBLK   # 16 v-tiles per block

_CACHE = {}
LAST_RESULT = None


def _build(pre, post):
    from concourse import bass, bacc, tile, mybir
    f32, bf16 = mybir.dt.float32, mybir.dt.bfloat16
    AF = mybir.ActivationFunctionType
    ts = bass.ts
    FUNCS = [AF.Identity, AF.Relu, AF.Gelu, AF.Square,
             AF.Identity, AF.Abs, AF.Tanh, AF.Sigmoid]

    nc = bacc.Bacc("TRN2", target_bir_lowering=False, debug=False,
                   num_devices=NCORES)
    # x pre-tiled on host: [128, NTC*NBLK*VB*512]; block (tcn,blk) is a
    # contiguous [128, VB*512] slab.
    xh = nc.dram_tensor("xh", [128, NTC * NBLK * VB * 512], bf16,
                        kind="ExternalInput")
    rw = nc.dram_tensor("rw", [128, NV * C], bf16, kind="ExternalInput")
    wwT = nc.dram_tensor("wwT", [C, V], bf16, kind="ExternalInput")
    opw = nc.dram_tensor("opw", [C, NOPS * C], bf16, kind="ExternalInput")
    opb = nc.dram_tensor("opb", [C, NOPS], f32, kind="ExternalInput")
    out = nc.dram_tensor("out", [T, V], bf16, kind="ExternalOutput")
    # out rows blocked by 128-partition groups for 2-row (2MB) stores
    out_r = out.ap().rearrange("(r p) v -> p r v", p=128)

    with tile.TileContext(nc) as tc:
        with tc.tile_pool(name="const", bufs=1) as constp, \
             tc.tile_pool(name="xt", bufs=4) as xtp, \
             tc.tile_pool(name="vals_ps", bufs=2, space="PSUM") as vpsp, \
             tc.tile_pool(name="vals_sb", bufs=2) as vsbp, \
             tc.tile_pool(name="h_ps", bufs=2, space="PSUM") as hpsp, \
             tc.tile_pool(name="t_sb", bufs=3) as tp, \
             tc.tile_pool(name="acc", bufs=2) as accp, \
             tc.tile_pool(name="accb", bufs=2) as accbp, \
             tc.tile_pool(name="out_ps", bufs=2, space="PSUM") as opsp, \
             tc.tile_pool(name="out_sb", bufs=2) as osbp:

            # consts on the ACT HWDGE ring so x loads (SP ring) start at t=0
            rw_t = constp.tile([128, NV * C], bf16)
            nc.scalar.dma_start(rw_t[:], rw.ap())
            wwT_t = constp.tile([C, V], bf16)
            nc.scalar.dma_start(wwT_t[:], wwT.ap())
            opw_t = constp.tile([C, NOPS * C], bf16)
            nc.scalar.dma_start(opw_t[:], opw.ap())
            opb_t = constp.tile([C, NOPS], f32)
            nc.scalar.dma_start(opb_t[:], opb.ap())

            for tcn in range(NTC):
                # read: accumulate over all V into one psum bank
                values = vpsp.tile([128, 512], f32)
                for blk in range(NBLK):
                    xt = xtp.tile([128, VB * 512], bf16)
                    nc.sync.dma_start(
                        xt[:], xh.ap()[:, ts(tcn * NBLK + blk, VB * 512)])
                    for j in range(VB):
                        vt = blk * VB + j
                        nc.tensor.matmul(values[:], rw_t[:, ts(vt, C)],
                                         xt[:, ts(j, 512)],
                                         start=(vt == 0), stop=(vt == NV - 1))
                vals = vsbp.tile([128, 512], bf16)
                nc.vector.tensor_copy(vals[:], values[:])

                # op bank: h_i -> ACT nonlin -> weighted accumulate in f32
                acc = accp.tile([128, 512], f32)
                for i in range(NOPS):
                    h = hpsp.tile([128, 512], f32)
                    nc.tensor.matmul(h[:], opw_t[:, ts(i, C)], vals[:],
                                     start=True, stop=True)
                    if i == 0:
                        nc.scalar.activation(acc[:], h[:], FUNCS[0],
                                             bias=opb_t[:, 0:1], scale=pre[0])
                    else:
                        t = tp.tile([128, 512], f32)
                        nc.scalar.activation(t[:], h[:], FUNCS[i],
                                             bias=opb_t[:, i:i + 1], scale=pre[i])
                        nc.vector.scalar_tensor_tensor(
                            acc[:], t[:], post[i], acc[:],
                            op0=mybir.AluOpType.mult, op1=mybir.AluOpType.add)
                accb = accbp.tile([128, 512], bf16)
                nc.vector.tensor_copy(accb[:], acc[:])

                # write: 2-bank psum tiles, 1024-wide drains, 2-row stores
                for sub2 in range(2):
                    osb = osbp.tile([128, 2, V], bf16)
                    for s in range(2):
                        sub = sub2 * 2 + s
                        for nn2 in range(4):
                            ops2 = opsp.tile([128, 1024], f32)
                            nc.tensor.matmul(ops2[:, 0:512],
                                             accb[:, ts(sub, 128)],
                                             wwT_t[:, ts(nn2 * 2, 512)],
                                             start=True, stop=True)
                            nc.tensor.matmul(ops2[:, 512:1024],
                                             accb[:, ts(sub, 128)],
                                             wwT_t[:, ts(nn2 * 2 + 1, 512)],
                                             start=True, stop=True)
                            idx = (tcn * 4 + sub) * 4 + nn2
                            if idx % 4 == 3:   # ~1/4 of psum drains go to ACT
                                nc.scalar.copy(osb[:, s, ts(nn2, 1024)],
                                               ops2[:])
                            else:
                                nc.vector.tensor_copy(osb[:, s, ts(nn2, 1024)],
                                                      ops2[:])
                    nc.gpsimd.dma_start(
                        out_r[:, ts(tcn * 2 + sub2, 2), :], osb[:])
    nc.compile()
    return nc


def _softmax(x, axis):
    x = np.asarray(x, np.float32)
    m = x.max(axis=axis, keepdims=True)
    e = np.exp(x - m)
    return e / e.sum(axis=axis, keepdims=True)


def kernel(x, basis, read_coeffs, write_coeffs, op_logits, op_weights,
           op_biases, out_scale):
    global LAST_RESULT
    import ml_dtypes
    from concourse.bass_utils import run_bass_kernel_spmd
    bf16 = ml_dtypes.bfloat16

    x = np.asarray(x, np.float32)
    basis = np.asarray(basis, np.float32)
    read_coeffs = np.asarray(read_coeffs, np.float32)
    write_coeffs = np.asarray(write_coeffs, np.float32)
    op_logits = np.asarray(op_logits, np.float32)
    op_weights = np.asarray(op_weights, np.float32)
    op_biases = np.asarray(op_biases, np.float32)
    out_scale = np.float32(out_scale)

    read_w = _softmax(basis @ read_coeffs.T, axis=0)               # (V, C)
    wwT = np.ascontiguousarray((basis @ write_coeffs.T).T) * out_scale  # (C, V)
    w = _softmax(op_logits, axis=0).astype(np.float64)

    # fold the mixture weight into ACT scale/bias where the nonlinearity allows
    #   i: 0 ident, 1 relu, 2 gelu, 3 square, 4 neg, 5 abs, 6 tanh, 7 sigmoid
    pre = [w[0], w[1], 1.0, np.sqrt(w[3]), -w[4], w[5], 1.0, 1.0]
    post = [1.0, 1.0, w[2], 1.0, 1.0, 1.0, w[6], w[7]]
    pre = [float(v) for v in pre]
    post = [float(v) for v in post]

    key = tuple(pre) + tuple(post)
    if key not in _CACHE:
        _CACHE[key] = _build(pre, post)
    nc = _CACHE[key]

    opb = (op_biases.T * np.array(pre, np.float64)[None, :]).astype(np.float32)
    # gelu/tanh/sigmoid biases enter before the nonlinearity unscaled
    for i in (2, 6, 7):
        opb[:, i] = op_biases[i]

    # rw: (V, C) -> [p, vt, c];  opw: (NOPS, C, C) -> [p, i, c]
    rwH = np.ascontiguousarray(
        read_w.reshape(NV, 128, C).transpose(1, 0, 2)).reshape(128, NV * C)
    opwH = np.ascontiguousarray(
        op_weights.transpose(1, 0, 2)).reshape(C, NOPS * C)

    shared = {
        "rw": rwH.astype(bf16),
        "wwT": wwT.astype(bf16),
        "opw": opwH.astype(bf16),
        "opb": np.ascontiguousarray(opb),
    }
    # x[b] (T, V) -> [p, tcn, blk, j, tt] with v = (blk*VB + j)*128 + p
    x16 = x.astype(bf16)
    in_maps = []
    for b in range(B):
        xb = x16[b].reshape(NTC, 512, NBLK, VB, 128).transpose(4, 0, 2, 3, 1)
        m = dict(shared)
        m["xh"] = np.ascontiguousarray(xb).reshape(128, NTC * NBLK * VB * 512)
        in_maps.append(m)

    res = run_bass_kernel_spmd(nc, in_maps, core_ids=list(range(NCORES)))
    LAST_RESULT = res
    out = np.empty((B, T, V), np.float32)
    for b in range(B):
        out[b] = np.asarray(res.results[b]["out"], np.float32)
    return out


# revision 4
# speedup vs baseline: 1.6963x; 1.0132x over previous
"""Trainium2 Bass kernel for the LGP-instruction module (read -> op bank -> write).

Data-parallel over batch: core b computes x[b] (2048, 4096).
All HBM traffic in bf16 (x, params, out) -- the problem is memory-bound, so
halving bytes halves the roofline; rel-err budget (2e-2) easily covers bf16.

Device pipeline per core, per T-chunk of 512 (software-pipelined so PE never
idles long enough for HAM to re-throttle):
  phase 1: values[C, 512] = sum_vt rw[vt].T @ x_tile[vt]    (PSUM f32, bf16 MMs)
  phase 2: h_k = W_k.T @ vals (PSUM) -> ACT f_k(h + b_k) -> DVE bf16 accumulate
           (identity and neg ops are pre-merged on host: 7 effective ops)
  phase 3: out[128T, V] = acc.T @ wwT  (PSUM f32) -> 1024-wide drains -> stores

Host prep: read_w softmax, write_w*out_scale transpose, mixture weights folded
into op weights/biases, x pre-tiled to the exact SBUF layout in bf16 so every
load is one contiguous descriptor set. Output returned bf16, upcast on host.
"""
import sys
import numpy as np

if '/opt/trn_rl_repo' not in sys.path:
    sys.path.insert(0, '/opt/trn_rl_repo')

B, T, V, C, NOPS = 8, 2048, 4096, 128, 8
NCORES = 8
NV = V // 128     # 32 v-tiles
NTC = T // 512    # 4 T-chunks
NBLK = 2          # x load blocks per T-chunk (2MB each)
VB = NV // NBLK   # 16 v-tiles per block
NK = 7            # effective ops after identity+neg merge
ACT_SET = {1, 3, 6, 8, 10, 13, 15}   # 7/16 psum drains to ACT

_CACHE = {}
LAST_RESULT = None


def _build(post):
    from concourse import bass, bacc, tile, mybir
    f32, bf16 = mybir.dt.float32, mybir.dt.bfloat16
    AF = mybir.ActivationFunctionType
    ts = bass.ts
    FUNCS = [AF.Identity, AF.Relu, AF.Gelu, AF.Square,
             AF.Abs, AF.Tanh, AF.Sigmoid]

    nc = bacc.Bacc("TRN2", target_bir_lowering=False, debug=False,
                   num_devices=NCORES)
    xh = nc.dram_tensor("xh", [128, NTC * NBLK * VB * 512], bf16,
                        kind="ExternalInput")
    rw = nc.dram_tensor("rw", [128, NV * C], bf16, kind="ExternalInput")
    wwT = nc.dram_tensor("wwT", [C, V], bf16, kind="ExternalInput")
    opw = nc.dram_tensor("opw", [C, NK * C], bf16, kind="ExternalInput")
    opb = nc.dram_tensor("opb", [C, NK], f32, kind="ExternalInput")
    out = nc.dram_tensor("out", [T, V], bf16, kind="ExternalOutput")
    out_r = out.ap().rearrange("(r p) v -> p r v", p=128)

    with tile.TileContext(nc) as tc:
        with tc.tile_pool(name="const", bufs=1) as constp, \
             tc.tile_pool(name="xt", bufs=4) as xtp, \
             tc.tile_pool(name="vals_ps", bufs=2, space="PSUM") as vpsp, \
             tc.tile_pool(name="vals_sb", bufs=2) as vsbp, \
             tc.tile_pool(name="h_ps", bufs=2, space="PSUM") as hpsp, \
             tc.tile_pool(name="t_sb", bufs=3) as tp, \
             tc.tile_pool(name="acc", bufs=2) as accp, \
             tc.tile_pool(name="out_ps", bufs=2, space="PSUM") as opsp, \
             tc.tile_pool(name="out_sb", bufs=2) as osbp:

            # rw first on the SP ring (needed by the very first matmul);
            # remaining consts go via the ACT HWDGE ring so x loads aren't
            # queued behind them.
            rw_t = constp.tile([128, NV * C], bf16)
            nc.sync.dma_start(rw_t[:], rw.ap())
            opb_t = constp.tile([C, NK], f32)
            nc.scalar.dma_start(opb_t[:], opb.ap())
            opw_t = constp.tile([C, NK * C], bf16)
            nc.scalar.dma_start(opw_t[:], opw.ap())
            wwT_t = constp.tile([C, V], bf16)
            nc.scalar.dma_start(wwT_t[:], wwT.ap())

            vals_sb = [None] * NTC

            def phase1(tcn):
                # read: accumulate over all V into one psum bank
                values = vpsp.tile([128, 512], f32)
                if tcn == 0:
                    # split the first block so the first MMs start earlier
                    blocks = [(0, VB // 2), (VB // 2, VB // 2), (VB, VB)]
                else:
                    blocks = [(0, VB), (VB, VB)]
                for v0, nvb in blocks:
                    xt = xtp.tile([128, nvb * 512], bf16)
                    start_el = (tcn * NV + v0) * 512
                    assert start_el % (nvb * 512) == 0
                    nc.sync.dma_start(
                        xt[:], xh.ap()[:, ts(start_el // (nvb * 512),
                                             nvb * 512)])
                    for j in range(nvb):
                        vt = v0 + j
                        nc.tensor.matmul(values[:], rw_t[:, ts(vt, C)],
                                         xt[:, ts(j, 512)],
                                         start=(vt == 0), stop=(vt == NV - 1))
                vals = vsbp.tile([128, 512], bf16)
                nc.vector.tensor_copy(vals[:], values[:])
                vals_sb[tcn] = vals

            def opbank(tcn):
                vals = vals_sb[tcn]
                acc = accp.tile([128, 512], bf16)
                for k in range(NK):
                    h = hpsp.tile([128, 512], f32)
                    nc.tensor.matmul(h[:], opw_t[:, ts(k, C)], vals[:],
                                     start=True, stop=True)
                    if k == 0:
                        nc.scalar.activation(acc[:], h[:], FUNCS[0],
                                             bias=opb_t[:, 0:1])
                    else:
                        t = tp.tile([128, 512], bf16)
                        nc.scalar.activation(t[:], h[:], FUNCS[k],
                                             bias=opb_t[:, k:k + 1])
                        nc.vector.scalar_tensor_tensor(
                            acc[:], t[:], post[k], acc[:],
                            op0=mybir.AluOpType.mult, op1=mybir.AluOpType.add)
                return acc

            def write(tcn, acc):
                for sub2 in range(2):
                    osb = osbp.tile([128, 2, V], bf16)
                    for s in range(2):
                        sub = sub2 * 2 + s
                        for nn2 in range(4):
                            ops2 = opsp.tile([128, 1024], f32)
                            nc.tensor.matmul(ops2[:, 0:512],
                                             acc[:, ts(sub, 128)],
                                             wwT_t[:, ts(nn2 * 2, 512)],
                                             start=True, stop=True)
                            nc.tensor.matmul(ops2[:, 512:1024],
                                             acc[:, ts(sub, 128)],
                                             wwT_t[:, ts(nn2 * 2 + 1, 512)],
                                             start=True, stop=True)
                            if (sub * 4 + nn2) in ACT_SET:
                                nc.scalar.copy(osb[:, s, ts(nn2, 1024)],
                                               ops2[:])
                            else:
                                nc.vector.tensor_copy(osb[:, s, ts(nn2, 1024)],
                                                      ops2[:])
                    nc.gpsimd.dma_start(
                        out_r[:, ts(tcn * 2 + sub2, 2), :], osb[:])

            phase1(0)
            for c in range(NTC):
                acc = opbank(c)
                if c + 1 < NTC:
                    phase1(c + 1)
                write(c, acc)
    nc.compile()
    return nc


def _softmax(x, axis):
    x = np.asarray(x, np.float32)
    m = x.max(axis=axis, keepdims=True)
    e = np.exp(x - m)
    return e / e.sum(axis=axis, keepdims=True)


def kernel(x, basis, read_coeffs, write_coeffs, op_logits, op_weights,
           op_biases, out_scale):
    global LAST_RESULT
    import ml_dtypes
    from concourse.bass_utils import run_bass_kernel_spmd
    bf16 = ml_dtypes.bfloat16

    x = np.asarray(x, np.float32)
    basis = np.asarray(basis, np.float32)
    read_coeffs = np.asarray(read_coeffs, np.float32)
    write_coeffs = np.asarray(write_coeffs, np.float32)
    op_logits = np.asarray(op_logits, np.float32)
    op_weights = np.asarray(op_weights, np.float64)
    op_biases = np.asarray(op_biases, np.float64)
    out_scale = np.float32(out_scale)

    read_w = _softmax(basis @ read_coeffs.T, axis=0)               # (V, C)
    wwT = np.ascontiguousarray((basis @ write_coeffs.T).T) * out_scale  # (C, V)
    w = _softmax(op_logits, axis=0).astype(np.float64)

    # Fold mixture weights into op weights/biases where the nonlinearity
    # allows; merge the two linear ops (identity, neg) into one.
    #   orig i: 0 ident, 1 relu, 2 gelu, 3 square, 4 neg, 5 abs, 6 tanh, 7 sigm
    Wm = [w[0] * op_weights[0] - w[4] * op_weights[4],
          w[1] * op_weights[1],
          op_weights[2],
          np.sqrt(w[3]) * op_weights[3],
          w[5] * op_weights[5],
          op_weights[6],
          op_weights[7]]
    bm = [w[0] * op_biases[0] - w[4] * op_biases[4],
          w[1] * op_biases[1],
          op_biases[2],
          np.sqrt(w[3]) * op_biases[3],
          w[5] * op_biases[5],
          op_biases[6],
          op_biases[7]]
    post = [1.0, 1.0, float(w[2]), 1.0, 1.0, float(w[6]), float(w[7])]

    key = tuple(post) + (float(w[0]), float(w[4]))
    if key not in _CACHE:
        _CACHE[key] = _build(post)
    nc = _CACHE[key]

    opw_eff = np.stack(Wm).astype(np.float32)          # (NK, C, C)
    opb_eff = np.stack(bm).astype(np.float32).T        # (C, NK)

    # rw: (V, C) -> [p, vt, c];  opw: (NK, C, C) -> [p, k, c]
    rwH = np.ascontiguousarray(
        read_w.reshape(NV, 128, C).transpose(1, 0, 2)).reshape(128, NV * C)
    opwH = np.ascontiguousarray(
        opw_eff.transpose(1, 0, 2)).reshape(C, NK * C)

    shared = {
        "rw": rwH.astype(bf16),
        "wwT": wwT.astype(bf16),
        "opw": opwH.astype(bf16),
        "opb": np.ascontiguousarray(opb_eff),
    }
    # x[b] (T, V) -> [p, tcn, blk, j, tt] with v = (blk*VB + j)*128 + p
    x16 = x.astype(bf16)
    in_maps = []
    for b in range(B):
        xb = x16[b].reshape(NTC, 512, NBLK, VB, 128).transpose(4, 0, 2, 3, 1)
        m = dict(shared)
        m["xh"] = np.ascontiguousarray(xb).reshape(128, NTC * NBLK * VB * 512)
        in_maps.append(m)

    res = run_bass_kernel_spmd(nc, in_maps, core_ids=list(range(NCORES)))
    LAST_RESULT = res
    out = np.empty((B, T, V), np.float32)
    for b in range(B):
        out[b] = np.asarray(res.results[b]["out"], np.float32)
    return out


# revision 5
# speedup vs baseline: 1.9500x; 1.1495x over previous
"""Trainium2 Bass kernel for the LGP-instruction module (read -> op bank -> write).

Data-parallel over batch: core b computes x[b] (2048, 4096).
All HBM traffic in bf16 (x, params, out) -- the problem is memory-bound, so
halving bytes halves the roofline; rel-err budget (2e-2) easily covers bf16.

Device pipeline per core, per T-chunk of 512 (software-pipelined so PE never
idles long enough for HAM to re-throttle):
  phase 1: values[C, 512] = sum_vt rw[vt].T @ x_tile[vt]    (PSUM f32, bf16 MMs)
  phase 2: h_k = W_k.T @ vals (PSUM) -> ACT f_k(h + b_k) -> DVE bf16 accumulate
           (identity and neg ops are pre-merged on host: 7 effective ops)
  phase 3: out[128T, V] = acc.T @ wwT  (PSUM f32) -> 1024-wide drains -> stores

Host prep: read_w softmax, write_w*out_scale transpose, mixture weights folded
into op weights/biases, x pre-tiled to the exact SBUF layout in bf16 so every
load is one contiguous descriptor set. Output returned bf16, upcast on host.
"""
import sys
import numpy as np

if '/opt/trn_rl_repo' not in sys.path:
    sys.path.insert(0, '/opt/trn_rl_repo')

B, T, V, C, NOPS = 8, 2048, 4096, 128, 8
NCORES = 8
NV = V // 128     # 32 v-tiles
NTC = T // 512    # 4 T-chunks
NBLK = 2          # x load blocks per T-chunk (2MB each)
VB = NV // NBLK   # 16 v-tiles per block
NK = 7            # effective ops after identity+neg merge
ACT_SET = {1, 3, 6, 8, 10, 13, 15}   # 7/16 psum drains to ACT

_CACHE = {}
LAST_RESULT = None


def _build(post):
    from concourse import bass, bacc, tile, mybir
    f32, bf16 = mybir.dt.float32, mybir.dt.bfloat16
    fp8 = mybir.dt.float8e4
    AF = mybir.ActivationFunctionType
    ts = bass.ts
    FUNCS = [AF.Identity, AF.Relu, AF.Gelu, AF.Square,
             AF.Abs, AF.Tanh, AF.Sigmoid]

    nc = bacc.Bacc("TRN2", target_bir_lowering=False, debug=False,
                   num_devices=NCORES)
    xh = nc.dram_tensor("xh", [128, NTC * NBLK * VB * 512], fp8,
                        kind="ExternalInput")
    rw = nc.dram_tensor("rw", [128, NV * C], bf16, kind="ExternalInput")
    wwT = nc.dram_tensor("wwT", [C, V], bf16, kind="ExternalInput")
    opw = nc.dram_tensor("opw", [C, NK * C], bf16, kind="ExternalInput")
    opb = nc.dram_tensor("opb", [C, NK], f32, kind="ExternalInput")
    out = nc.dram_tensor("out", [T, V], bf16, kind="ExternalOutput")
    out_r = out.ap().rearrange("(r p) v -> p r v", p=128)

    with tile.TileContext(nc) as tc:
        with tc.tile_pool(name="const", bufs=1) as constp, \
             tc.tile_pool(name="xt", bufs=4) as xtp, \
             tc.tile_pool(name="vals_ps", bufs=2, space="PSUM") as vpsp, \
             tc.tile_pool(name="vals_sb", bufs=2) as vsbp, \
             tc.tile_pool(name="h_ps", bufs=2, space="PSUM") as hpsp, \
             tc.tile_pool(name="t_sb", bufs=3) as tp, \
             tc.tile_pool(name="acc", bufs=2) as accp, \
             tc.tile_pool(name="out_ps", bufs=2, space="PSUM") as opsp, \
             tc.tile_pool(name="out_sb", bufs=4) as osbp:

            # rw first on the SP ring (needed by the very first matmul);
            # remaining consts go via the ACT HWDGE ring so x loads aren't
            # queued behind them.
            rw_t = constp.tile([128, NV * C], bf16)
            for q in range(4):
                nc.sync.dma_start(rw_t[:, ts(q, NV * C // 4)],
                                  rw.ap()[:, ts(q, NV * C // 4)])
            opb_t = constp.tile([C, NK], f32)
            nc.scalar.dma_start(opb_t[:], opb.ap())
            opw_t = constp.tile([C, NK * C], bf16)
            nc.scalar.dma_start(opw_t[:], opw.ap())
            wwT_t = constp.tile([C, V], bf16)
            nc.scalar.dma_start(wwT_t[:], wwT.ap())

            vals_sb = [None] * NTC

            def phase1(tcn):
                # read: accumulate over all V into one psum bank
                values = vpsp.tile([128, 512], f32)
                if tcn == 0:
                    # split the first block so the first MMs start earlier
                    blocks = [(0, VB // 2), (VB // 2, VB // 2), (VB, VB)]
                else:
                    blocks = [(0, VB), (VB, VB)]
                for v0, nvb in blocks:
                    xt = xtp.tile([128, nvb * 512], fp8)
                    start_el = (tcn * NV + v0) * 512
                    assert start_el % (nvb * 512) == 0
                    nc.sync.dma_start(
                        xt[:], xh.ap()[:, ts(start_el // (nvb * 512),
                                             nvb * 512)])
                    for j in range(nvb):
                        vt = v0 + j
                        nc.tensor.matmul(values[:], rw_t[:, ts(vt, C)],
                                         xt[:, ts(j, 512)],
                                         start=(vt == 0), stop=(vt == NV - 1))
                vals = vsbp.tile([128, 512], bf16)
                nc.vector.tensor_copy(vals[:], values[:])
                vals_sb[tcn] = vals

            def opbank(tcn):
                vals = vals_sb[tcn]
                acc = accp.tile([128, 512], bf16)
                for k in range(NK):
                    h = hpsp.tile([128, 512], f32)
                    nc.tensor.matmul(h[:], opw_t[:, ts(k, C)], vals[:],
                                     start=True, stop=True)
                    if k == 0:
                        nc.scalar.activation(acc[:], h[:], FUNCS[0],
                                             bias=opb_t[:, 0:1])
                    else:
                        t = tp.tile([128, 512], bf16)
                        nc.scalar.activation(t[:], h[:], FUNCS[k],
                                             bias=opb_t[:, k:k + 1])
                        nc.vector.scalar_tensor_tensor(
                            acc[:], t[:], post[k], acc[:],
                            op0=mybir.AluOpType.mult, op1=mybir.AluOpType.add)
                return acc

            def write(tcn, acc):
                for sub2 in range(2):
                    osb = osbp.tile([128, 2, V], bf16)
                    for s in range(2):
                        sub = sub2 * 2 + s
                        for nn2 in range(4):
                            ops2 = opsp.tile([128, 1024], f32)
                            nc.tensor.matmul(ops2[:, 0:512],
                                             acc[:, ts(sub, 128)],
                                             wwT_t[:, ts(nn2 * 2, 512)],
                                             start=True, stop=True)
                            nc.tensor.matmul(ops2[:, 512:1024],
                                             acc[:, ts(sub, 128)],
                                             wwT_t[:, ts(nn2 * 2 + 1, 512)],
                                             start=True, stop=True)
                            if (sub * 4 + nn2) in ACT_SET:
                                nc.scalar.copy(osb[:, s, ts(nn2, 1024)],
                                               ops2[:])
                            else:
                                nc.vector.tensor_copy(osb[:, s, ts(nn2, 1024)],
                                                      ops2[:])
                    nc.gpsimd.dma_start(
                        out_r[:, ts(tcn * 2 + sub2, 2), :], osb[:])

            phase1(0)
            for c in range(NTC):
                acc = opbank(c)
                if c + 1 < NTC:
                    phase1(c + 1)
                write(c, acc)
    nc.compile()
    return nc


def _softmax(x, axis):
    x = np.asarray(x, np.float32)
    m = x.max(axis=axis, keepdims=True)
    e = np.exp(x - m)
    return e / e.sum(axis=axis, keepdims=True)


def kernel(x, basis, read_coeffs, write_coeffs, op_logits, op_weights,
           op_biases, out_scale):
    global LAST_RESULT
    import ml_dtypes
    from concourse.bass_utils import run_bass_kernel_spmd
    bf16 = ml_dtypes.bfloat16

    x = np.asarray(x, np.float32)
    basis = np.asarray(basis, np.float32)
    read_coeffs = np.asarray(read_coeffs, np.float32)
    write_coeffs = np.asarray(write_coeffs, np.float32)
    op_logits = np.asarray(op_logits, np.float32)
    op_weights = np.asarray(op_weights, np.float64)
    op_biases = np.asarray(op_biases, np.float64)
    out_scale = np.float32(out_scale)

    read_w = _softmax(basis @ read_coeffs.T, axis=0)               # (V, C)
    wwT = np.ascontiguousarray((basis @ write_coeffs.T).T) * out_scale  # (C, V)
    w = _softmax(op_logits, axis=0).astype(np.float64)

    # Fold mixture weights into op weights/biases where the nonlinearity
    # allows; merge the two linear ops (identity, neg) into one.
    #   orig i: 0 ident, 1 relu, 2 gelu, 3 square, 4 neg, 5 abs, 6 tanh, 7 sigm
    Wm = [w[0] * op_weights[0] - w[4] * op_weights[4],
          w[1] * op_weights[1],
          op_weights[2],
          np.sqrt(w[3]) * op_weights[3],
          w[5] * op_weights[5],
          op_weights[6],
          op_weights[7]]
    bm = [w[0] * op_biases[0] - w[4] * op_biases[4],
          w[1] * op_biases[1],
          op_biases[2],
          np.sqrt(w[3]) * op_biases[3],
          w[5] * op_biases[5],
          op_biases[6],
          op_biases[7]]
    post = [1.0, 1.0, float(w[2]), 1.0, 1.0, float(w[6]), float(w[7])]

    key = tuple(post) + (float(w[0]), float(w[4]))
    if key not in _CACHE:
        _CACHE[key] = _build(post)
    nc = _CACHE[key]

    opw_eff = np.stack(Wm).astype(np.float32)          # (NK, C, C)
    opb_eff = np.stack(bm).astype(np.float32).T        # (C, NK)

    # rw: (V, C) -> [p, vt, c];  opw: (NK, C, C) -> [p, k, c]
    rwH = np.ascontiguousarray(
        read_w.reshape(NV, 128, C).transpose(1, 0, 2)).reshape(128, NV * C)
    opwH = np.ascontiguousarray(
        opw_eff.transpose(1, 0, 2)).reshape(C, NK * C)

    shared = {
        "rw": rwH.astype(bf16),
        "wwT": wwT.astype(bf16),
        "opw": opwH.astype(bf16),
        "opb": np.ascontiguousarray(opb_eff),
    }
    # x[b] (T, V) -> [p, tcn, blk, j, tt] with v = (blk*VB + j)*128 + p
    x16 = x.astype(ml_dtypes.float8_e4m3)
    in_maps = []
    for b in range(B):
        xb = x16[b].reshape(NTC, 512, NBLK, VB, 128).transpose(4, 0, 2, 3, 1)
        m = dict(shared)
        m["xh"] = np.ascontiguousarray(xb).reshape(128, NTC * NBLK * VB * 512)
        in_maps.append(m)

    res = run_bass_kernel_spmd(nc, in_maps, core_ids=list(range(NCORES)))
    LAST_RESULT = res
    out = np.empty((B, T, V), np.float32)
    for b in range(B):
        out[b] = np.asarray(res.results[b]["out"], np.float32)
    return out


# revision 7
# speedup vs baseline: 2.0028x; 1.0271x over previous
"""Trainium2 Bass kernel for the LGP-instruction module (read -> op bank -> write).

Data-parallel over batch: core b computes x[b] (2048, 4096).
All HBM traffic in bf16 (x, params, out) -- the problem is memory-bound, so
halving bytes halves the roofline; rel-err budget (2e-2) easily covers bf16.

Device pipeline per core, per T-chunk of 512 (software-pipelined so PE never
idles long enough for HAM to re-throttle):
  phase 1: values[C, 512] = sum_vt rw[vt].T @ x_tile[vt]    (PSUM f32, bf16 MMs)
  phase 2: h_k = W_k.T @ vals (PSUM) -> ACT f_k(h + b_k) -> DVE bf16 accumulate
           (identity and neg ops are pre-merged on host: 7 effective ops)
  phase 3: out[128T, V] = acc.T @ wwT  (PSUM f32) -> 1024-wide drains -> stores

Host prep: read_w softmax, write_w*out_scale transpose, mixture weights folded
into op weights/biases, x pre-tiled to the exact SBUF layout in bf16 so every
load is one contiguous descriptor set. Output returned bf16, upcast on host.
"""
import sys
import numpy as np

if '/opt/trn_rl_repo' not in sys.path:
    sys.path.insert(0, '/opt/trn_rl_repo')

B, T, V, C, NOPS = 8, 2048, 4096, 128, 8
NCORES = 8
NV = V // 128     # 32 v-tiles
NTC = T // 512    # 4 T-chunks
NBLK = 2          # x load blocks per T-chunk (2MB each)
VB = NV // NBLK   # 16 v-tiles per block
NK = 7            # effective ops after identity+neg merge
ACT_SET = {1, 3, 6, 8, 10, 13, 15}   # 7/16 psum drains to ACT

_CACHE = {}
LAST_RESULT = None


def _build(post):
    from concourse import bass, bacc, tile, mybir
    f32, bf16 = mybir.dt.float32, mybir.dt.bfloat16
    fp8 = mybir.dt.float8e4
    AF = mybir.ActivationFunctionType
    ts = bass.ts
    FUNCS = [AF.Identity, AF.Relu, AF.Gelu, AF.Square,
             AF.Abs, AF.Tanh, AF.Sigmoid]

    nc = bacc.Bacc("TRN2", target_bir_lowering=False, debug=False,
                   num_devices=NCORES)
    xh = nc.dram_tensor("xh", [128, NTC * NBLK * VB * 512], fp8,
                        kind="ExternalInput")
    rw = nc.dram_tensor("rw", [128, NV * C], bf16, kind="ExternalInput")
    wwT = nc.dram_tensor("wwT", [C, V], bf16, kind="ExternalInput")
    opw = nc.dram_tensor("opw", [C, NK * C], bf16, kind="ExternalInput")
    opb = nc.dram_tensor("opb", [C, NK], f32, kind="ExternalInput")
    out = nc.dram_tensor("out", [T, V], bf16, kind="ExternalOutput")
    out_r = out.ap().rearrange("(r p) v -> p r v", p=128)

    with tile.TileContext(nc) as tc:
        with tc.tile_pool(name="const", bufs=1) as constp, \
             tc.tile_pool(name="xt", bufs=4) as xtp, \
             tc.tile_pool(name="vals_ps", bufs=2, space="PSUM") as vpsp, \
             tc.tile_pool(name="vals_sb", bufs=2) as vsbp, \
             tc.tile_pool(name="h_ps", bufs=2, space="PSUM") as hpsp, \
             tc.tile_pool(name="t_sb", bufs=3) as tp, \
             tc.tile_pool(name="acc", bufs=2) as accp, \
             tc.tile_pool(name="out_ps", bufs=2, space="PSUM") as opsp, \
             tc.tile_pool(name="out_sb", bufs=4) as osbp:

            # rw first on the SP ring (needed by the very first matmul);
            # remaining consts go via the ACT HWDGE ring so x loads aren't
            # queued behind them.
            rw_t = constp.tile([128, NV * C], bf16)
            nc.sync.dma_start(rw_t[:, ts(0, NV * C // 4)],
                              rw.ap()[:, ts(0, NV * C // 4)])
            opb_t = constp.tile([C, NK], f32)
            nc.scalar.dma_start(opb_t[:], opb.ap())
            opw_t = constp.tile([C, NK * C], bf16)
            nc.scalar.dma_start(opw_t[:], opw.ap())
            wwT_t = constp.tile([C, V], bf16)
            nc.scalar.dma_start(wwT_t[:], wwT.ap())

            vals_sb = [None] * NTC

            def phase1(tcn):
                # read: accumulate over all V into one psum bank
                values = vpsp.tile([128, 512], f32)
                if tcn == 0:
                    # split the first blocks so the first MMs start earlier
                    blocks = [(0, 4), (4, 4), (8, 8), (16, 16)]
                else:
                    blocks = [(0, VB), (VB, VB)]
                for bi, (v0, nvb) in enumerate(blocks):
                    xt = xtp.tile([128, nvb * 512], fp8)
                    start_el = (tcn * NV + v0) * 512
                    assert start_el % (nvb * 512) == 0
                    nc.sync.dma_start(
                        xt[:], xh.ap()[:, ts(start_el // (nvb * 512),
                                             nvb * 512)])
                    if tcn == 0 and bi == 0:
                        # rw pieces 1-3 queue behind the first x piece
                        for q in range(1, 4):
                            nc.sync.dma_start(
                                rw_t[:, ts(q, NV * C // 4)],
                                rw.ap()[:, ts(q, NV * C // 4)])
                    for j in range(nvb):
                        vt = v0 + j
                        nc.tensor.matmul(values[:], rw_t[:, ts(vt, C)],
                                         xt[:, ts(j, 512)],
                                         start=(vt == 0), stop=(vt == NV - 1))
                vals = vsbp.tile([128, 512], bf16)
                nc.vector.tensor_copy(vals[:], values[:])
                vals_sb[tcn] = vals

            def opbank(tcn):
                vals = vals_sb[tcn]
                acc = accp.tile([128, 512], bf16)
                for k in range(NK):
                    h = hpsp.tile([128, 512], f32)
                    nc.tensor.matmul(h[:], opw_t[:, ts(k, C)], vals[:],
                                     start=True, stop=True)
                    if k == 0:
                        nc.scalar.activation(acc[:], h[:], FUNCS[0],
                                             bias=opb_t[:, 0:1])
                    else:
                        t = tp.tile([128, 512], bf16)
                        nc.scalar.activation(t[:], h[:], FUNCS[k],
                                             bias=opb_t[:, k:k + 1])
                        nc.vector.scalar_tensor_tensor(
                            acc[:], t[:], post[k], acc[:],
                            op0=mybir.AluOpType.mult, op1=mybir.AluOpType.add)
                return acc

            def write(tcn, acc):
                for sub2 in range(2):
                    osb = osbp.tile([128, 2, V], bf16)
                    for s in range(2):
                        sub = sub2 * 2 + s
                        for nn2 in range(4):
                            ops2 = opsp.tile([128, 1024], f32)
                            nc.tensor.matmul(ops2[:, 0:512],
                                             acc[:, ts(sub, 128)],
                                             wwT_t[:, ts(nn2 * 2, 512)],
                                             start=True, stop=True)
                            nc.tensor.matmul(ops2[:, 512:1024],
                                             acc[:, ts(sub, 128)],
                                             wwT_t[:, ts(nn2 * 2 + 1, 512)],
                                             start=True, stop=True)
                            if (sub * 4 + nn2) in ACT_SET:
                                nc.scalar.copy(osb[:, s, ts(nn2, 1024)],
                                               ops2[:])
                            else:
                                nc.vector.tensor_copy(osb[:, s, ts(nn2, 1024)],
                                                      ops2[:])
                    nc.gpsimd.dma_start(
                        out_r[:, ts(tcn * 2 + sub2, 2), :], osb[:])

            phase1(0)
            for c in range(NTC):
                acc = opbank(c)
                if c + 1 < NTC:
                    phase1(c + 1)
                write(c, acc)
    nc.compile()
    return nc


def _softmax(x, axis):
    x = np.asarray(x, np.float32)
    m = x.max(axis=axis, keepdims=True)
    e = np.exp(x - m)
    return e / e.sum(axis=axis, keepdims=True)


def kernel(x, basis, read_coeffs, write_coeffs, op_logits, op_weights,
           op_biases, out_scale):
    global LAST_RESULT
    import ml_dtypes
    from concourse.bass_utils import run_bass_kernel_spmd
    bf16 = ml_dtypes.bfloat16

    x = np.asarray(x, np.float32)
    basis = np.asarray(basis, np.float32)
    read_coeffs = np.asarray(read_coeffs, np.float32)
    write_coeffs = np.asarray(write_coeffs, np.float32)
    op_logits = np.asarray(op_logits, np.float32)
    op_weights = np.asarray(op_weights, np.float64)
    op_biases = np.asarray(op_biases, np.float64)
    out_scale = np.float32(out_scale)

    read_w = _softmax(basis @ read_coeffs.T, axis=0)               # (V, C)
    wwT = np.ascontiguousarray((basis @ write_coeffs.T).T) * out_scale  # (C, V)
    w = _softmax(op_logits, axis=0).astype(np.float64)

    # Fold mixture weights into op weights/biases where the nonlinearity
    # allows; merge the two linear ops (identity, neg) into one.
    #   orig i: 0 ident, 1 relu, 2 gelu, 3 square, 4 neg, 5 abs, 6 tanh, 7 sigm
    Wm = [w[0] * op_weights[0] - w[4] * op_weights[4],
          w[1] * op_weights[1],
          op_weights[2],
          np.sqrt(w[3]) * op_weights[3],
          w[5] * op_weights[5],
          op_weights[6],
          op_weights[7]]
    bm = [w[0] * op_biases[0] - w[4] * op_biases[4],
          w[1] * op_biases[1],
          op_biases[2],
          np.sqrt(w[3]) * op_biases[3],
          w[5] * op_biases[5],
          op_biases[6],
          op_biases[7]]
    post = [1.0, 1.0, float(w[2]), 1.0, 1.0, float(w[6]), float(w[7])]

    key = tuple(post) + (float(w[0]), float(w[4]))
    if key not in _CACHE:
        _CACHE[key] = _build(post)
    nc = _CACHE[key]

    opw_eff = np.stack(Wm).astype(np.float32)          # (NK, C, C)
    opb_eff = np.stack(bm).astype(np.float32).T        # (C, NK)

    # rw: (V, C) -> [p, vt, c];  opw: (NK, C, C) -> [p, k, c]
    rwH = np.ascontiguousarray(
        read_w.reshape(NV, 128, C).transpose(1, 0, 2)).reshape(128, NV * C)
    opwH = np.ascontiguousarray(
        opw_eff.transpose(1, 0, 2)).reshape(C, NK * C)

    shared = {
        "rw": rwH.astype(bf16),
        "wwT": wwT.astype(bf16),
        "opw": opwH.astype(bf16),
        "opb": np.ascontiguousarray(opb_eff),
    }
    # x[b] (T, V) -> [p, tcn, blk, j, tt] with v = (blk*VB + j)*128 + p
    x16 = x.astype(ml_dtypes.float8_e4m3)
    in_maps = []
    for b in range(B):
        xb = x16[b].reshape(NTC, 512, NBLK, VB, 128).transpose(4, 0, 2, 3, 1)
        m = dict(shared)
        m["xh"] = np.ascontiguousarray(xb).reshape(128, NTC * NBLK * VB * 512)
        in_maps.append(m)

    res = run_bass_kernel_spmd(nc, in_maps, core_ids=list(range(NCORES)))
    LAST_RESULT = res
    out = np.empty((B, T, V), np.float32)
    for b in range(B):
        out[b] = np.asarray(res.results[b]["out"], np.float32)
    return out
